# revision 3
# baseline (speedup 1.0000x reference)
"""Trainium2 Bass kernel for nn_Classifier_custom_12936441496172.

Reference math (per batch b, with av = column-l2-normalized img_b [Cf, R]):
    A      = softmax_r( (vv @ W1) @ av )          # [I, R] attention over R
    F_p    = A @ av.T                             # [I, Cf]
    out[b] = rowsum( (vv @ W2) * F_p )            # [I]

Key identity used here: out[b, i] = sum_r A[i, r] * ((vv @ W2) @ av)[i, r],
so the big F_p intermediate is never materialized. Both (vv@W1)@av and
(vv@W2)@av come from one stacked weight matrix QPT = concat(Q, P).T, and the
column normalization of av folds into a per-column scale of the matmul
output: (Q @ av)[i, r] = (Q @ img_b)[i, r] * rn[r], rn = 1/||img_b[:, r]||.

Sharding: data-parallel over batch across 8 NeuronCores (16 batches each),
with the small parameter matrix QPT replicated. Parameter prep (vv @ W1/W2,
< 0.1% of total FLOPs) happens on host; all img-dependent compute (norms,
main matmuls, softmax, weighted dots) runs on-device.

Device kernel per core: 8 groups of 2 batches (N = 512 matmul free dim):
  - norm: bf16 squares (ACT/DVE) + ones-vector matmul accumulating over the
    8 K-chunks -> n2 [1, 512] in PSUM; rn = sqrt(1/n2); gpsimd broadcast to
    all 128 partitions.
  - main: per m-chunk of the 624 stacked rows, 8 accumulating float32r
    matmuls (full PE rate at N >= 256) -> S [m, 512] in PSUM.
  - softmax+dot: S_Q*rn (DVE, reads PSUM), Exp with free per-partition
    accum (ACT) -> sumexp, reciprocal, then one fused DVE
    scalar_tensor_tensor (E * recip) * S_P with accum -> output column.
Logits are ~N(0,1) (|logit| < ~6) so the softmax max-subtraction is skipped;
exp cannot overflow fp32.
"""

import numpy as np

_PROGRAM = None

# Problem geometry (hardcoded per contract; kernel.py must be self-contained)
N_CORES = 8
NB = 16          # batches per core
R = 256          # H * W
CF = 1024        # feature channels
KC = CF // 128   # 8 contraction chunks
I = 312          # attributes
G = NB // 2      # groups of 2 batches
N = 2 * R        # matmul moving free dim (2 batches)
MCH = [(0, 128), (128, 128), (256, 56)]  # m-chunks of the 312 rows


def _build_program():
    from contextlib import ExitStack

    import concourse.tile as tile
    from concourse import bacc, mybir

    F32 = mybir.dt.float32
    F32R = mybir.dt.float32r
    BF16 = mybir.dt.bfloat16
    MULT = mybir.AluOpType.mult
    EXP = mybir.ActivationFunctionType.Exp

    nc = bacc.Bacc(
        "TRN2",
        target_bir_lowering=False,
        debug=False,
        enable_asserts=False,
        num_devices=N_CORES,
    )
    img = nc.dram_tensor("img", [NB, CF, R], F32R, kind="ExternalInput").ap()
    qpt = nc.dram_tensor("qpt", [CF, 2 * I], F32R, kind="ExternalInput").ap()
    out = nc.dram_tensor("out", [I, NB], F32, kind="ExternalOutput").ap()

    with tile.TileContext(nc) as tc, tc.tile_pool(name="sb", bufs=2) as sb, tc.tile_pool(
        name="ps", bufs=4, space="PSUM"
    ) as ps:
        # Replicated stationary weights: one resident tile, [128, 624] per k.
        qpt_sb = sb.tile([128, KC * 2 * I], F32R, tag="qpt", bufs=1, name="qpt_sb")
        for k in range(KC):
            nc.sync.dma_start(
                qpt_sb[:, k * 2 * I : (k + 1) * 2 * I], qpt[k * 128 : (k + 1) * 128, :]
            )
        ones_col = nc.const_aps.tensor(1.0, (128, 1), BF16)

        outsb = [
            sb.tile([msz, NB], F32, tag=f"out{mi}", bufs=1, name=f"outsb{mi}")
            for mi, (_, msz) in enumerate(MCH)
        ]

        def load_x(g):
            xs = []
            for k in range(KC):
                x = sb.tile([128, N], F32R, tag=f"x{k}", bufs=2, name=f"x{k}g{g}")
                for h in range(2):
                    nc.sync.dma_start(
                        x[:, h * R : (h + 1) * R],
                        img[2 * g + h, k * 128 : (k + 1) * 128, :],
                    )
                xs.append(x)
            return xs

        def norm_chain(g, xs):
            # n2[r] = sum_f x[f, r]^2: bf16 squares, summed over partitions
            # by an accumulating ones-vector matmul.
            n2 = ps.tile([1, N], F32, tag="n2", bufs=2, name=f"n2g{g}")
            for k in range(KC):
                sq = sb.tile([128, N], BF16, tag="sq", bufs=3, name=f"sqg{g}k{k}")
                if k % 2 == 0:
                    nc.scalar.square(sq[:], xs[k][:].bitcast(F32))
                else:
                    nc.vector.tensor_mul(sq[:], xs[k][:].bitcast(F32), xs[k][:].bitcast(F32))
                nc.tensor.matmul(
                    n2[:], ones_col, sq[:], start=(k == 0), stop=(k == KC - 1)
                )
            inv = sb.tile([1, N], F32, tag="inv", bufs=2, name=f"invg{g}")
            nc.vector.reciprocal(inv[:], n2[:])
            rnr = sb.tile([1, N], F32, tag="rnr", bufs=2, name=f"rnrg{g}")
            nc.scalar.sqrt(rnr[:], inv[:])  # rn = sqrt(1/n2) = 1/||x_r||
            rn = sb.tile([128, N], F32, tag="rn", bufs=2, name=f"rng{g}")
            nc.gpsimd.partition_broadcast(rn[:], rnr[:], channels=128)
            return rn

        def main_group(g, xs, rn):
            for mi, (moff, msz) in enumerate(MCH):
                acc = []
                for side in range(2):  # 0 = Q (logits), 1 = P
                    off = side * I + moff
                    a = ps.tile(
                        [msz, N], F32, tag="sps", bufs=4, name=f"accg{g}m{mi}s{side}"
                    )
                    for k in range(KC):
                        nc.tensor.matmul(
                            a[:],
                            qpt_sb[:, k * 2 * I + off : k * 2 * I + off + msz],
                            xs[k][:],
                            start=(k == 0),
                            stop=(k == KC - 1),
                        )
                    acc.append(a)
                sqs = sb.tile([msz, N], F32, tag="sqs", bufs=2, name=f"sqsg{g}m{mi}")
                nc.vector.tensor_mul(sqs[:], acc[0][:], rn[:msz, :])
                sps = sb.tile([msz, N], F32, tag="spss", bufs=2, name=f"spsg{g}m{mi}")
                nc.vector.tensor_mul(sps[:], acc[1][:], rn[:msz, :])
                E = sb.tile([msz, N], F32, tag="E", bufs=2, name=f"Eg{g}m{mi}")
                se = sb.tile([msz, 2], F32, tag="se", bufs=2, name=f"seg{g}m{mi}")
                for h in range(2):
                    nc.scalar.activation(
                        E[:, h * R : (h + 1) * R],
                        sqs[:, h * R : (h + 1) * R],
                        EXP,
                        accum_out=se[:, h : h + 1],
                    )
                rec = sb.tile([msz, 2], F32, tag="rec", bufs=2, name=f"recg{g}m{mi}")
                nc.vector.reciprocal(rec[:], se[:])
                scr = sb.tile([msz, R], F32, tag="scr", bufs=2, name=f"scrg{g}m{mi}")
                for h in range(2):
                    nc.vector.scalar_tensor_tensor(
                        out=scr[:],
                        in0=E[:, h * R : (h + 1) * R],
                        scalar=rec[:, h : h + 1],
                        in1=sps[:, h * R : (h + 1) * R],
                        op0=MULT,
                        op1=MULT,
                        accum_out=outsb[mi][:, 2 * g + h : 2 * g + h + 1],
                    )

        # Software-pipelined emission: prefetch next group's inputs and norm
        # chain around the current group's main matmuls.
        xs_cur = load_x(0)
        rn_cur = norm_chain(0, xs_cur)
        for g in range(G):
            if g + 1 < G:
                xs_nxt = load_x(g + 1)
            main_group(g, xs_cur, rn_cur)
            if g + 1 < G:
                rn_cur = norm_chain(g + 1, xs_nxt)
                xs_cur = xs_nxt
        for mi, (moff, msz) in enumerate(MCH):
            nc.sync.dma_start(out[moff : moff + msz, :], outsb[mi][:])

    nc.compile()
    return nc


def _prepare(inputs):
    img = np.asarray(inputs["img"], np.float32)
    V = np.asarray(inputs["V"], np.float32)
    W1 = np.asarray(inputs["W1"], np.float32)
    W2 = np.asarray(inputs["W2"], np.float32)
    B, Cf, H, W = img.shape
    assert (B, Cf, H * W) == (N_CORES * NB, CF, R), img.shape

    vv = V.astype(np.float64)
    vv /= np.maximum(np.sqrt((vv * vv).sum(1, keepdims=True)), 1e-12)
    qpt = np.concatenate(
        [vv @ W1.astype(np.float64), vv @ W2.astype(np.float64)], axis=0
    ).T
    qpt = np.ascontiguousarray(qpt, np.float32)  # [CF, 624]

    imgr = img.reshape(B, Cf, H * W)
    in_maps = [
        {"img": np.ascontiguousarray(imgr[c * NB : (c + 1) * NB]), "qpt": qpt}
        for c in range(N_CORES)
    ]
    return in_maps


def run(inputs, **spmd_kwargs):
    """Run the kernel; returns (full_output [B, I], BassKernelResults)."""
    global _PROGRAM
    if _PROGRAM is None:
        _PROGRAM = _build_program()
    from concourse.bass_utils import run_bass_kernel_spmd

    in_maps = _prepare(inputs)
    res = run_bass_kernel_spmd(
        _PROGRAM, in_maps, core_ids=list(range(N_CORES)), **spmd_kwargs
    )
    out = np.concatenate(
        [np.asarray(res.results[c]["out"]).T for c in range(N_CORES)], axis=0
    )
    return np.ascontiguousarray(out, np.float32), res


def kernel(**inputs) -> np.ndarray:
    return run(inputs)[0]


# revision 4
# speedup vs baseline: 1.0808x; 1.0808x over previous
"""Trainium2 Bass kernel for nn_Classifier_custom_12936441496172.

Reference math (per batch b, with av = column-l2-normalized img_b [Cf, R]):
    A      = softmax_r( (vv @ W1) @ av )          # [I, R] attention over R
    F_p    = A @ av.T                             # [I, Cf]
    out[b] = rowsum( (vv @ W2) * F_p )            # [I]

Key identity used here: out[b, i] = sum_r A[i, r] * ((vv @ W2) @ av)[i, r],
so the big F_p intermediate is never materialized. Both (vv@W1)@av and
(vv@W2)@av come from one stacked weight matrix QPT, and the column
normalization of av folds into a per-column scale of the matmul output:
(Q @ av)[i, r] = (Q @ img_b)[i, r] * rn[r], rn = 1/||img_b[:, r]||.

Sharding: data-parallel over batch across 8 NeuronCores (16 batches each),
with the small parameter matrix QPT replicated. Parameter prep (vv @ W1/W2,
< 1% of total FLOPs) happens on host; all img-dependent compute (norms,
main matmuls, softmax, weighted dots) runs on-device.

Device kernel per core: 8 groups of 2 batches (N = 512 matmul free dim):
  - norms: fp16 squares (ACT/DVE) + fp16 pair-add tree (DVE 2x mode) +
    gpsimd partition_all_reduce -> n2 broadcast on all partitions;
    rn = Exp(-0.5 * Ln(n2)) on ACT (square/ln/exp share one ACT table set,
    so no table reloads).
  - main: 5 m-chunks of the 624 stacked rows (Q0, Q1, P0, P1, QP-tail
    packed into one 112-row chunk via host-side column reorder), each 8
    accumulating float32r matmuls (full PE rate at N >= 256).
  - softmax+dot: S*rn (DVE, reads PSUM), Exp with free per-partition
    accum (ACT) -> sumexp matrix, then one fused DVE scalar_tensor_tensor
    E * S_P with free-axis accum -> unnormalized output column. The
    softmax denominator is applied once per core at the end (3 tiny
    reciprocal+multiply ops on [*, 16] tiles).
Logits are ~N(0,1) (|logit| < ~6) so the softmax max-subtraction is skipped;
exp cannot overflow fp32.
"""

import numpy as np

_PROGRAM = None

# Problem geometry (hardcoded per contract; kernel.py must be self-contained)
N_CORES = 8
NB = 16          # batches per core
R = 256          # H * W
CF = 1024        # feature channels
KC = CF // 128   # 8 contraction chunks
I = 312          # attributes
G = NB // 2      # groups of 2 batches
N = 2 * R        # matmul moving free dim (2 batches)
TQ = I - 256     # 56-row tails
# m-chunk column offsets in the host-reordered qpt
MCH_Q = [0, 128]       # Q rows 0:128, 128:256
MCH_P = [256, 384]     # P rows 0:128, 128:256
MCH_T = 512            # Q rows 256:312 at cols 512:568, P rows at 568:624
# number of squares computed on ACT (rest on DVE)
SQ_ON_ACT = 6


def _build_program():
    import concourse.tile as tile
    from concourse import bacc, bass_isa, mybir

    F32 = mybir.dt.float32
    F32R = mybir.dt.float32r
    F16 = mybir.dt.float16
    MULT = mybir.AluOpType.mult
    EXP = mybir.ActivationFunctionType.Exp
    LN = mybir.ActivationFunctionType.Ln

    nc = bacc.Bacc(
        "TRN2",
        target_bir_lowering=False,
        debug=False,
        enable_asserts=False,
        num_devices=N_CORES,
    )
    img = nc.dram_tensor("img", [NB, CF, R], F32R, kind="ExternalInput").ap()
    qpt = nc.dram_tensor("qpt", [CF, 2 * I], F32R, kind="ExternalInput").ap()
    out = nc.dram_tensor("out", [I, NB], F32, kind="ExternalOutput").ap()

    with tile.TileContext(nc) as tc, tc.tile_pool(name="sb", bufs=2) as sb, tc.tile_pool(
        name="ps", bufs=5, space="PSUM"
    ) as ps:
        qpt_sb = sb.tile([128, KC * 2 * I], F32R, tag="qpt", bufs=1, name="qpt_sb")
        for k in range(KC):
            nc.sync.dma_start(
                qpt_sb[:, k * 2 * I : (k + 1) * 2 * I], qpt[k * 128 : (k + 1) * 128, :]
            )

        # Persistent per-core accumulators: unnormalized dots + sumexp matrix.
        MSZ = [128, 128, TQ]
        outsb = [
            sb.tile([msz, NB], F32, tag=f"out{mi}", bufs=1, name=f"outsb{mi}")
            for mi, msz in enumerate(MSZ)
        ]
        semat = [
            sb.tile([msz, NB], F32, tag=f"se{mi}", bufs=1, name=f"semat{mi}")
            for mi, msz in enumerate(MSZ)
        ]

        def load_x(g):
            xs = []
            for k in range(KC):
                x = sb.tile([128, N], F32R, tag=f"x{k}", bufs=2, name=f"x{k}g{g}")
                for h in range(2):
                    nc.sync.dma_start(
                        x[:, h * R : (h + 1) * R],
                        img[2 * g + h, k * 128 : (k + 1) * 128, :],
                    )
                xs.append(x)
            return xs

        def norm_chain(g, xs):
            # n2[r] = sum_f x[f, r]^2, on all partitions: fp16 squares ->
            # fp16 pair-add tree (DVE 2x) -> gpsimd partition all-reduce.
            sq = []
            for k in range(KC):
                s = sb.tile([128, N], F16, tag=f"sq{k % 4}", bufs=2, name=f"sqg{g}k{k}")
                if k < SQ_ON_ACT:
                    nc.scalar.square(s[:], xs[k][:].bitcast(F32))
                else:
                    nc.vector.tensor_mul(
                        s[:], xs[k][:].bitcast(F32), xs[k][:].bitcast(F32)
                    )
                sq.append(s)
            lvl, li = sq, 0
            while len(lvl) > 1:
                nxt = []
                for j in range(0, len(lvl), 2):
                    t = sb.tile(
                        [128, N], F16, tag=f"ss{li}{j}", bufs=2, name=f"ssg{g}l{li}j{j}"
                    )
                    nc.vector.tensor_add(t[:], lvl[j][:], lvl[j + 1][:])
                    nxt.append(t)
                lvl, li = nxt, li + 1
            n2 = sb.tile([128, N], F32, tag="n2", bufs=2, name=f"n2g{g}")
            nc.gpsimd.partition_all_reduce(
                n2[:], lvl[0][:], channels=128, reduce_op=bass_isa.ReduceOp.add
            )
            # rn = n2^(-1/2) = Exp(-0.5 * Ln(n2)); Ln/Exp/Square share an ACT
            # function set -> no activation table reloads.
            lnt = sb.tile([128, N], F32, tag="lnt", bufs=2, name=f"lntg{g}")
            nc.scalar.activation(lnt[:], n2[:], LN)
            rn = sb.tile([128, N], F32, tag="rn", bufs=2, name=f"rng{g}")
            nc.scalar.activation(rn[:], lnt[:], EXP, scale=-0.5)
            return rn

        def mm_chunk(g, xs, coff, msz, nm):
            a = ps.tile([msz, N], F32, tag="sps", bufs=5, name=f"ps{nm}g{g}")
            for k in range(KC):
                nc.tensor.matmul(
                    a[:],
                    qpt_sb[:, k * 2 * I + coff : k * 2 * I + coff + msz],
                    xs[k][:],
                    start=(k == 0),
                    stop=(k == KC - 1),
                )
            return a

        def softmax_dot(g, mi, sqs, sps, msz):
            # sqs: scaled Q-side logits [msz, N]; sps: scaled P-side [msz, N].
            E = sb.tile([msz, N], F32, tag="E", bufs=2, name=f"Eg{g}m{mi}")
            for h in range(2):
                nc.scalar.activation(
                    E[:, h * R : (h + 1) * R],
                    sqs[:, h * R : (h + 1) * R],
                    EXP,
                    accum_out=semat[mi][:msz, 2 * g + h : 2 * g + h + 1],
                )
            scr = sb.tile([msz, R], F32, tag="scr", bufs=2, name=f"scrg{g}m{mi}")
            for h in range(2):
                nc.vector.scalar_tensor_tensor(
                    out=scr[:],
                    in0=E[:, h * R : (h + 1) * R],
                    scalar=1.0,
                    in1=sps[:, h * R : (h + 1) * R],
                    op0=MULT,
                    op1=MULT,
                    accum_out=outsb[mi][:msz, 2 * g + h : 2 * g + h + 1],
                )

        def main_group(g, xs, rn):
            # Two full 128-row chunk pairs.
            for mi in range(2):
                qa = mm_chunk(g, xs, MCH_Q[mi], 128, f"q{mi}")
                pa = mm_chunk(g, xs, MCH_P[mi], 128, f"p{mi}")
                sqs = sb.tile([128, N], F32, tag="sqs", bufs=2, name=f"sqsg{g}m{mi}")
                nc.vector.tensor_mul(sqs[:], qa[:], rn[:, :])
                sps = sb.tile([128, N], F32, tag="spss", bufs=2, name=f"spsg{g}m{mi}")
                nc.vector.tensor_mul(sps[:], pa[:], rn[:, :])
                softmax_dot(g, mi, sqs, sps, 128)
            # Packed tail: Q rows 256:312 at psum partitions 0:56, P rows at 56:112.
            ta = mm_chunk(g, xs, MCH_T, 2 * TQ, "t")
            ts = sb.tile([2 * TQ, N], F32, tag="tss", bufs=2, name=f"tsg{g}")
            nc.vector.tensor_mul(ts[:], ta[:], rn[: 2 * TQ, :])
            # Shift the P half down to partitions 0:56 (DMA, split over 2 queues).
            tp = sb.tile([TQ, N], F32, tag="tps", bufs=2, name=f"tpg{g}")
            hh = TQ // 2
            nc.sync.dma_start(tp[:hh, :], ts[TQ : TQ + hh, :])
            nc.sync.dma_start(tp[hh:, :], ts[TQ + hh :, :])
            softmax_dot(g, 2, ts[:TQ, :], tp[:], TQ)

        xs_cur = load_x(0)
        rn_cur = norm_chain(0, xs_cur)
        for g in range(G):
            if g + 1 < G:
                xs_nxt = load_x(g + 1)
            main_group(g, xs_cur, rn_cur)
            if g + 1 < G:
                rn_cur = norm_chain(g + 1, xs_nxt)
                xs_cur = xs_nxt

        # Final softmax normalization + store.
        offs = [0, 128, 256]
        for mi, msz in enumerate(MSZ):
            rec = sb.tile([msz, NB], F32, tag=f"rec{mi}", bufs=1, name=f"rec{mi}")
            nc.vector.reciprocal(rec[:], semat[mi][:])
            fin = sb.tile([msz, NB], F32, tag=f"fin{mi}", bufs=1, name=f"fin{mi}")
            nc.vector.tensor_mul(fin[:], outsb[mi][:], rec[:])
            nc.sync.dma_start(out[offs[mi] : offs[mi] + msz, :], fin[:])

    nc.compile()
    return nc


def _prepare(inputs):
    img = np.asarray(inputs["img"], np.float32)
    V = np.asarray(inputs["V"], np.float32)
    W1 = np.asarray(inputs["W1"], np.float32)
    W2 = np.asarray(inputs["W2"], np.float32)
    B, Cf, H, W = img.shape
    assert (B, Cf, H * W) == (N_CORES * NB, CF, R), img.shape

    vv = V.astype(np.float64)
    vv /= np.maximum(np.sqrt((vv * vv).sum(1, keepdims=True)), 1e-12)
    Q = vv @ W1.astype(np.float64)  # [I, CF]
    P = vv @ W2.astype(np.float64)
    # Column order: Q[0:128], Q[128:256], P[0:128], P[128:256], Q[256:], P[256:]
    stacked = np.concatenate(
        [Q[0:128], Q[128:256], P[0:128], P[128:256], Q[256:I], P[256:I]], axis=0
    )
    qpt = np.ascontiguousarray(stacked.T, np.float32)  # [CF, 624]

    imgr = img.reshape(B, Cf, H * W)
    in_maps = [
        {"img": np.ascontiguousarray(imgr[c * NB : (c + 1) * NB]), "qpt": qpt}
        for c in range(N_CORES)
    ]
    return in_maps


def run(inputs, **spmd_kwargs):
    """Run the kernel; returns (full_output [B, I], BassKernelResults)."""
    global _PROGRAM
    if _PROGRAM is None:
        _PROGRAM = _build_program()
    from concourse.bass_utils import run_bass_kernel_spmd

    in_maps = _prepare(inputs)
    res = run_bass_kernel_spmd(
        _PROGRAM, in_maps, core_ids=list(range(N_CORES)), **spmd_kwargs
    )
    out = np.concatenate(
        [np.asarray(res.results[c]["out"]).T for c in range(N_CORES)], axis=0
    )
    return np.ascontiguousarray(out, np.float32), res


def kernel(**inputs) -> np.ndarray:
    return run(inputs)[0]
